# revision 1
# baseline (speedup 1.0000x reference)
"""Deformable conv (3x3, offsets from a conv) + GroupNorm(32) + ReLU on 8 Trainium2 cores.

Sharding: (batch=4) x (row-halves=2) -> 8 cores. Each core computes one
half-sample (2048 output positions). GroupNorm statistics are AllReduce'd
(add) between the two cores of each sample.

Per-core pipeline:
  1. offset conv (bf16 PE matmuls over a host-padded input window)
  2. PE-transpose offsets to position-major; DVE computes bilinear corner
     indices + validity-masked weights (all fp32, +16-shifted coords)
  3. dma_gather from a DRAM token table x_t[4096, 256] (bf16), fetching
     x-adjacent pairs (elem_step=256, elem_size=512) per y-corner
  4. DVE combines the 4 corners with free-broadcast weights -> cols (bf16)
  5. PE-transposes cols into K-major matmul layout; 2304-deep bf16 matmul
  6. GroupNorm stats (ACT accum) -> pair AllReduce -> fused scale/bias ReLU
"""
import sys

sys.path.insert(0, "/opt/trn_rl_repo/concourse")
sys.path.insert(0, "/opt/trn_rl_repo")

import numpy as np
import ml_dtypes

import concourse.bass as bass
import concourse.bacc as bacc
import concourse.tile as tile
import concourse.mybir as mybir
from concourse.bass_utils import run_bass_kernel_spmd

F32 = mybir.dt.float32
BF16 = mybir.dt.bfloat16
I16 = mybir.dt.int16
AOT = mybir.AluOpType
ACTF = mybir.ActivationFunctionType

B, C, H, W = 4, 256, 64, 64
KK = 3
P9 = 9
G = 32
EPS = 1e-5
N_CORES = 8
NPOS = 2048          # positions per core (32 rows x 64 cols)
NJ = 16              # 128-position tiles per core
SH = 16.0            # coordinate shift so mod/floor operate on positives


def _ident(nc, name, dt):
    from concourse.masks import make_identity
    t = nc.alloc_sbuf_tensor(name, [128, 128], dt)
    make_identity(nc, t.ap())
    return t.ap()


def build_nc(with_collective=True, debug_outs=False):
    nc = bacc.Bacc("TRN2", target_bir_lowering=False, debug=False,
                   num_devices=N_CORES if with_collective else 1)
    DIV = 32768.0 if with_collective else 16384.0

    # ---------------- I/O ----------------
    xt = nc.dram_tensor("xt", [4096, 256], BF16, kind="ExternalInput")
    xconv = nc.dram_tensor("xconv", [2, 128, 34, 66], F32, kind="ExternalInput").ap()
    wofft = nc.dram_tensor("wofft", [128, 2, 9, 18], F32, kind="ExternalInput").ap()
    wt = nc.dram_tensor("wt", [128, 2, 9, 256], F32, kind="ExternalInput").ap()
    cy = nc.dram_tensor("cy", [128, 16, 9], F32, kind="ExternalInput").ap()
    cx = nc.dram_tensor("cx", [128, 16, 9], F32, kind="ExternalInput").ap()
    bvec = nc.dram_tensor("bvec", [128, 2], F32, kind="ExternalInput").ap()
    gamv = nc.dram_tensor("gamv", [128, 2], F32, kind="ExternalInput").ap()
    betv = nc.dram_tensor("betv", [128, 2], F32, kind="ExternalInput").ap()
    boff = nc.dram_tensor("boff", [18, 1], F32, kind="ExternalInput").ap()
    ggrp = nc.dram_tensor("ggrp", [128, 16], F32, kind="ExternalInput").ap()
    gbrd = nc.dram_tensor("gbrd", [16, 128], F32, kind="ExternalInput").ap()
    yout = nc.dram_tensor("yout", [2, 128, 2048], F32, kind="ExternalOutput").ap()
    dbg = {}
    if debug_outs:
        dbg["off"] = nc.dram_tensor("dbg_off", [18, 2048], F32, kind="ExternalOutput").ap()
        dbg["wgt"] = nc.dram_tensor("dbg_wgt", [128, 16, 9, 4], F32, kind="ExternalOutput").ap()
        dbg["idx"] = nc.dram_tensor("dbg_idx", [128, 2304], I16, kind="ExternalOutput").ap()
        dbg["rhs"] = nc.dram_tensor("dbg_rhs", [128, 18, 2048], F32, kind="ExternalOutput").ap()

    with tile.TileContext(nc) as tc:
        # persistent sbuf tensors
        sb_wofft = nc.alloc_sbuf_tensor("sb_wofft", [128, 2, 9, 18], BF16).ap()
        sb_wt = nc.alloc_sbuf_tensor("sb_wt", [128, 2, 9, 256], BF16).ap()
        sb_cy = nc.alloc_sbuf_tensor("sb_cy", [128, 16, 9], F32).ap()
        sb_cx = nc.alloc_sbuf_tensor("sb_cx", [128, 16, 9], F32).ap()
        sb_bvec = nc.alloc_sbuf_tensor("sb_bvec", [128, 2], F32).ap()
        sb_gam = nc.alloc_sbuf_tensor("sb_gam", [128, 2], F32).ap()
        sb_bet = nc.alloc_sbuf_tensor("sb_bet", [128, 2], F32).ap()
        sb_boff = nc.alloc_sbuf_tensor("sb_boff", [18, 1], F32).ap()
        sb_ggrp = nc.alloc_sbuf_tensor("sb_ggrp", [128, 16], F32).ap()
        sb_gbrd = nc.alloc_sbuf_tensor("sb_gbrd", [16, 128], F32).ap()
        offv = nc.alloc_sbuf_tensor("offv", [128, 16, 18], F32).ap()
        wgt4 = nc.alloc_sbuf_tensor("wgt4", [128, 16, 9, 4], BF16)
        idxs16 = nc.alloc_sbuf_tensor("idxs16", [128, 18, 128], I16)
        rhs = nc.alloc_sbuf_tensor("rhs", [128, 18, 2048], BF16)
        sums = nc.alloc_sbuf_tensor("sums", [128, 2, 4], F32).ap()   # sum y per (o2, ntile)
        sumsq = nc.alloc_sbuf_tensor("sumsq", [128, 2, 4], F32).ap()
        allst = nc.alloc_sbuf_tensor("allst", [16, 2], F32).ap()
        allst2 = nc.alloc_sbuf_tensor("allst2", [16, 2], F32).ap()
        mr = nc.alloc_sbuf_tensor("mr", [16, 2], F32).ap()
        sb_eps = nc.alloc_sbuf_tensor("sb_eps", [16, 1], F32).ap()

        nc.vector.memset(sb_eps[:], EPS)
        id32 = _ident(nc, "id32", F32)
        id16 = _ident(nc, "id16", BF16)

        # ---------------- loads ----------------
        nc.gpsimd.dma_start(out=sb_wofft[:], in_=wofft)
        nc.gpsimd.dma_start(out=sb_wt[:], in_=wt)
        nc.sync.dma_start(out=sb_cy[:], in_=cy)
        nc.sync.dma_start(out=sb_cx[:], in_=cx)
        nc.sync.dma_start(out=sb_bvec[:], in_=bvec)
        nc.sync.dma_start(out=sb_gam[:], in_=gamv)
        nc.sync.dma_start(out=sb_bet[:], in_=betv)
        nc.sync.dma_start(out=sb_boff[:], in_=boff)
        nc.sync.dma_start(out=sb_ggrp[:], in_=ggrp)
        nc.sync.dma_start(out=sb_gbrd[:], in_=gbrd)

        with (
            tc.tile_pool(name="ps_small", bufs=2, space="PSUM") as pss,
            tc.tile_pool(name="ps_y", bufs=6, space="PSUM") as psy,
            tc.tile_pool(name="g", bufs=4) as gpool,
            tc.tile_pool(name="tmp", bufs=2) as tmppool,
            tc.tile_pool(name="acc", bufs=2) as accpool,
            tc.tile_pool(name="mth", bufs=2) as mth,
            tc.tile_pool(name="yo", bufs=2) as yopool,
            tc.tile_pool(name="dram", bufs=2, space="DRAM") as drpool,
        ):
            sb_xconv = gpool.tile([128, 2, 34, 66], BF16, tag="g", name="sb_xconv")
            off_sb = gpool.tile([18, 2048], F32, tag="g", name="off_sb")
            idxt_t = gpool.tile([18, 16, 128], F32, tag="g", name="idxt_t")
            nc.gpsimd.dma_start(out=sb_xconv[:], in_=bass.AP(
                tensor=xconv.tensor, offset=0,
                ap=[[34 * 66, 128], [128 * 34 * 66, 2], [66, 34], [1, 66]]))
            # ---------------- phase 1: offset conv ----------------
            for rc in range(4):  # 8-row chunks
                ps = pss.tile([18, 512], F32, tag="ps", space="PSUM")
                k = 0
                for c2 in range(2):
                    for t in range(9):
                        ky, kx = t // 3, t % 3
                        rv = sb_xconv[:, c2, rc * 8 + ky: rc * 8 + ky + 8, kx: kx + 64]
                        nc.tensor.matmul(ps[:], lhsT=sb_wofft[:, c2, t, :], rhs=rv,
                                         start=(k == 0), stop=(k == 17))
                        k += 1
                # += b_off while copying psum -> sbuf
                nc.vector.tensor_scalar(out=off_sb[:, rc * 512:(rc + 1) * 512],
                                        in0=ps[:], scalar1=sb_boff[:, 0:1],
                                        scalar2=None, op0=AOT.add)
            if debug_outs:
                nc.sync.dma_start(out=dbg["off"], in_=off_sb[:])

            # ---------------- phase 2: transpose offsets to [pos, 18] ----------------
            for j in range(NJ):
                pt = pss.tile([128, 18], F32, tag="ps", space="PSUM")
                nc.tensor.transpose(pt[:], off_sb[:, j * 128:(j + 1) * 128], id32[:18, :18])
                nc.vector.tensor_copy(out=offv[:, j, :], in_=pt[:])

            # ---------------- phase 3: coords / weights / indices ----------------
            _mtc = [0]

            def mt(shape=(128, 16, 9), dt=F32, tag="m"):
                _mtc[0] += 1
                return mth.tile(list(shape), dt, tag=tag, name=f"mt{_mtc[0]}", bufs=26 if tag == "m" else 2)

            dy = offv[:, :, 0:18:2]
            dx = offv[:, :, 1:18:2]
            ys, xs = mt(), mt()
            nc.vector.tensor_add(out=ys[:], in0=dy, in1=sb_cy[:])
            nc.vector.tensor_add(out=xs[:], in0=dx, in1=sb_cx[:])
            fy, fx, y0, x0 = mt(), mt(), mt(), mt()
            # floor(v) for v>0, robust to cast rounding mode: i=round(v);
            # floor = i - (i > v); frac = v - floor
            I32 = mybir.dt.int32
            for src_, fl_, fr_ in ((ys, y0, fy), (xs, x0, fx)):
                ic = mt(dt=I32, tag="mi")
                icf = mt()
                gt_ = mt()
                nc.vector.tensor_copy(out=ic[:], in_=src_[:])
                nc.vector.tensor_copy(out=icf[:], in_=ic[:])
                nc.vector.tensor_tensor(out=gt_[:], in0=icf[:], in1=src_[:], op=AOT.is_gt)
                nc.vector.tensor_tensor(out=fl_[:], in0=icf[:], in1=gt_[:], op=AOT.subtract)
                nc.vector.tensor_tensor(out=fr_[:], in0=src_[:], in1=fl_[:], op=AOT.subtract)
            wy0, wx0 = mt(), mt()
            nc.vector.tensor_scalar(out=wy0[:], in0=fy[:], scalar1=-1.0, scalar2=1.0,
                                    op0=AOT.mult, op1=AOT.add)
            nc.vector.tensor_scalar(out=wx0[:], in0=fx[:], scalar1=-1.0, scalar2=1.0,
                                    op0=AOT.mult, op1=AOT.add)

            def mask_in(src, lo, hi):
                g_, l_, m_ = mt(), mt(), mt()
                nc.vector.tensor_scalar(out=g_[:], in0=src[:], scalar1=lo, scalar2=None, op0=AOT.is_ge)
                nc.vector.tensor_scalar(out=l_[:], in0=src[:], scalar1=hi, scalar2=None, op0=AOT.is_le)
                nc.vector.tensor_tensor(out=m_[:], in0=g_[:], in1=l_[:], op=AOT.mult)
                return g_, l_, m_

            _, _, my0 = mask_in(y0, 16.0, 79.0)
            _, _, my1 = mask_in(y0, 15.0, 78.0)
            gx0, _, mx0 = mask_in(x0, 16.0, 79.0)
            _, lx62, mx1 = mask_in(x0, 15.0, 78.0)  # lx62: x0<=62

            wy0e, wy1e, wx0e, wx1e = mt(), mt(), mt(), mt()
            nc.vector.tensor_tensor(out=wy0e[:], in0=wy0[:], in1=my0[:], op=AOT.mult)
            nc.vector.tensor_tensor(out=wy1e[:], in0=fy[:], in1=my1[:], op=AOT.mult)
            nc.vector.tensor_tensor(out=wx0e[:], in0=wx0[:], in1=mx0[:], op=AOT.mult)
            nc.vector.tensor_tensor(out=wx1e[:], in0=fx[:], in1=mx1[:], op=AOT.mult)

            eqm1, eq63 = mt(), mt()
            nc.vector.tensor_scalar(out=eqm1[:], in0=x0[:], scalar1=15.0, scalar2=None, op0=AOT.is_equal)
            nc.vector.tensor_scalar(out=eq63[:], in0=x0[:], scalar1=79.0, scalar2=None, op0=AOT.is_equal)
            wxs0, wxs1, t1, t2 = mt(), mt(), mt(), mt()
            nc.vector.tensor_tensor(out=t1[:], in0=wx0e[:], in1=lx62[:], op=AOT.mult)
            nc.vector.tensor_tensor(out=t2[:], in0=wx1e[:], in1=eqm1[:], op=AOT.mult)
            nc.vector.tensor_tensor(out=wxs0[:], in0=t1[:], in1=t2[:], op=AOT.add)
            nc.vector.tensor_tensor(out=t1[:], in0=wx1e[:], in1=gx0[:], op=AOT.mult)
            nc.vector.tensor_tensor(out=t2[:], in0=wx0e[:], in1=eq63[:], op=AOT.mult)
            nc.vector.tensor_tensor(out=wxs1[:], in0=t1[:], in1=t2[:], op=AOT.add)

            w4 = wgt4.ap()
            nc.vector.tensor_tensor(out=w4[:, :, :, 0], in0=wy0e[:], in1=wxs0[:], op=AOT.mult)
            nc.vector.tensor_tensor(out=w4[:, :, :, 1], in0=wy0e[:], in1=wxs1[:], op=AOT.mult)
            nc.vector.tensor_tensor(out=w4[:, :, :, 2], in0=wy1e[:], in1=wxs0[:], op=AOT.mult)
            nc.vector.tensor_tensor(out=w4[:, :, :, 3], in0=wy1e[:], in1=wxs1[:], op=AOT.mult)
            if debug_outs:
                wdf = mt(tag="wdbg")
                for k4 in range(4):
                    nc.vector.tensor_copy(out=wdf[:], in_=w4[:, :, :, k4])
                    nc.sync.dma_start(out=dbg["wgt"][:, :, :, k4], in_=wdf[:])

            # indices (shifted coords; true idx = (yb-16)*64 + (xb-16))
            xb, yb0, yb1, idxf = mt(), mt(), mt(), mt(shape=(128, 16, 9, 2), tag="idxf")
            nc.vector.tensor_scalar(out=xb[:], in0=x0[:], scalar1=16.0, scalar2=78.0,
                                    op0=AOT.max, op1=AOT.min)
            nc.vector.tensor_scalar(out=yb0[:], in0=y0[:], scalar1=16.0, scalar2=79.0,
                                    op0=AOT.max, op1=AOT.min)
            nc.vector.tensor_scalar(out=yb1[:], in0=y0[:], scalar1=15.0, scalar2=78.0,
                                    op0=AOT.max, op1=AOT.min)
            nc.vector.scalar_tensor_tensor(out=t1[:], in0=yb0[:], scalar=64.0, in1=xb[:],
                                           op0=AOT.mult, op1=AOT.add)
            nc.vector.tensor_scalar(out=idxf[:, :, :, 0], in0=t1[:], scalar1=-1040.0, scalar2=None, op0=AOT.add)
            nc.vector.scalar_tensor_tensor(out=t2[:], in0=yb1[:], scalar=64.0, in1=xb[:],
                                           op0=AOT.mult, op1=AOT.add)
            nc.vector.tensor_scalar(out=idxf[:, :, :, 1], in0=t2[:], scalar1=-976.0, scalar2=None, op0=AOT.add)

            # ---------------- phase 4: idx transposes into wrapped layout ----------------
            # step 1: [128 pos, 18] -> idxt [18, 16 j, 128 pos] (f32)
            for j in range(NJ):
                pi = pss.tile([18, 128], F32, tag="ps", space="PSUM")
                nc.tensor.transpose(pi[:], idxf[:, j, :, :].rearrange("p t c -> p (t c)"),
                                    id32[:, :])
                nc.vector.tensor_copy(out=idxt_t[:, j, :], in_=pi[:])
            # step 2: fold pos%16 onto partitions: idxs16[r, tc, j*8+p16]
            #         = idxt[tc, j, 16*p16 + r], via [18,16]->[16,18] PE transposes
            for j in range(NJ):
                ptw = pss.tile([16, 8, 18], F32, tag="ps", space="PSUM")
                for p16 in range(8):
                    nc.tensor.transpose(ptw[:, p16, :],
                                        idxt_t[:, j, 16 * p16:16 * p16 + 16],
                                        id32[:18, :18])
                nc.vector.tensor_copy(out=idxs16.ap()[0:16, :, j * 8:(j + 1) * 8],
                                      in_=ptw[:].rearrange("r a t -> r t a"))
            for g8 in range(1, 8):
                nc.sync.dma_start(out=idxs16.ap()[g8 * 16:(g8 + 1) * 16, :, :],
                                  in_=idxs16.ap()[0:16, :, :])
            if debug_outs:
                nc.sync.dma_start(out=dbg["idx"], in_=idxs16.ap().rearrange("p a b -> p (a b)"))

            # ---------------- phases 5-6: gather + combine + transpose ----------------
            xt_view = bass.AP(tensor=xt, offset=0, ap=[[256, 4095], [1, 512]])
            for t9 in range(9):
                for jh in range(2):
                    gs = []
                    for yc in range(2):
                        g_ = gpool.tile([128, 8, 512], BF16, tag="g")
                        nc.gpsimd.dma_gather(
                            out_ap=g_[:], in_ap=xt_view,
                            idxs_ap=idxs16.ap()[:, t9 * 2 + yc, jh * 64:(jh + 1) * 64],
                            num_idxs=1024, num_idxs_reg=1024,
                            elem_size=512, elem_step=256)
                        gs.append(g_)
                    tmp0 = tmppool.tile([128, 8, 2, 256], BF16, tag="tt", bufs=3, name="tmp0")
                    tmp1 = tmppool.tile([128, 8, 2, 256], BF16, tag="tt", bufs=3, name="tmp1")
                    jsl = slice(jh * 8, (jh + 1) * 8)
                    w_y0 = w4[:, jsl, t9, 0:2].to_broadcast([128, 8, 2, 256])
                    w_y1 = w4[:, jsl, t9, 2:4].to_broadcast([128, 8, 2, 256])
                    g0v = gs[0][:].rearrange("p j (x c) -> p j x c", x=2)
                    g1v = gs[1][:].rearrange("p j (x c) -> p j x c", x=2)
                    nc.vector.tensor_tensor(out=tmp0[:], in0=g0v, in1=w_y0, op=AOT.mult)
                    nc.vector.tensor_tensor(out=tmp1[:], in0=g1v, in1=w_y1, op=AOT.mult)
                    nc.vector.tensor_tensor(out=tmp0[:], in0=tmp0[:], in1=tmp1[:], op=AOT.add)
                    if jh == 0:
                        acc = accpool.tile([128, 16, 256], BF16, tag="acc")
                    nc.vector.tensor_tensor(out=acc[:, jsl, :], in0=tmp0[:, :, 0, :],
                                            in1=tmp0[:, :, 1, :], op=AOT.add)
                # transpose this tap's cols into rhs
                for c2 in range(2):
                    ci = t9 * 2 + c2
                    for jb in range(4):
                        pt = pss.tile([128, 512], BF16, tag="ps", space="PSUM")
                        for jj in range(4):
                            nc.tensor.transpose(
                                pt[:, jj * 128:(jj + 1) * 128],
                                acc[:, jb * 4 + jj, c2 * 128:(c2 + 1) * 128],
                                id16[:, :])
                        nc.scalar.copy(out=rhs.ap()[:, ci, jb * 512:(jb + 1) * 512],
                                       in_=pt[:])
            if debug_outs:
                rdf = mth.tile([128, 2048], F32, tag="rdbg", bufs=1, name="rdf")
                for ci in range(18):
                    nc.vector.tensor_copy(out=rdf[:], in_=rhs.ap()[:, ci, :])
                    nc.sync.dma_start(out=dbg["rhs"][:, ci, :], in_=rdf[:])

            # ---------------- phases 7-11: matmul, GN stats, collective, norm ----------------
            scr = nc.alloc_sbuf_tensor("scr", [128, 512], BF16).ap()
            for o2 in range(2):
                ps_ys = []
                for nt in range(4):
                    ps_ys.append(psy.tile([128, 512], F32, tag="y", space="PSUM", name=f"psy{o2}_{nt}"))
                for ci in range(18):
                    t9, c2 = ci // 2, ci % 2
                    lt = sb_wt[:, c2, t9, o2 * 128:(o2 + 1) * 128]
                    for nt in range(4):
                        nc.tensor.matmul(ps_ys[nt][:], lhsT=lt,
                                         rhs=rhs.ap()[:, ci, nt * 512:(nt + 1) * 512],
                                         start=(ci == 0), stop=(ci == 17))
                # stats: sum and sum-of-squares per partition
                for nt in range(4):
                    nc.scalar.activation(out=scr[:], in_=ps_ys[nt][:], func=ACTF.Copy,
                                         accum_out=sums[:, o2, nt:nt + 1])
                    nc.scalar.activation(out=scr[:], in_=ps_ys[nt][:], func=ACTF.Square,
                                         accum_out=sumsq[:, o2, nt:nt + 1])
                sy = mth.tile([128, 1], F32, tag="s1", bufs=12, name="sy")
                qy = mth.tile([128, 1], F32, tag="s1", bufs=12)
                t1_ = mth.tile([128, 1], F32, tag="s1", bufs=12)
                t2_ = mth.tile([128, 1], F32, tag="s1", bufs=12)
                t3_ = mth.tile([128, 1], F32, tag="s1", bufs=12)
                nc.vector.reduce_sum(out=sy[:], in_=sums[:, o2, :],
                                     axis=mybir.AxisListType.X)
                nc.vector.reduce_sum(out=qy[:], in_=sumsq[:, o2, :],
                                     axis=mybir.AxisListType.X)
                b_ap = sb_bvec[:, o2:o2 + 1]
                # Sz = Sy + 2048*b ; Qz = Qy + 2*b*Sy + 2048*b^2
                st2 = mth.tile([128, 2], F32, tag="st2")
                nc.vector.scalar_tensor_tensor(out=st2[:, 0:1], in0=b_ap, scalar=2048.0,
                                               in1=sy[:], op0=AOT.mult, op1=AOT.add)
                nc.vector.scalar_tensor_tensor(out=t1_[:], in0=b_ap, scalar=2.0,
                                               in1=sy[:], op0=AOT.mult, op1=AOT.mult)
                nc.vector.scalar_tensor_tensor(out=t2_[:], in0=b_ap, scalar=2048.0,
                                               in1=b_ap, op0=AOT.mult, op1=AOT.mult)
                nc.vector.tensor_tensor(out=t3_[:], in0=t1_[:], in1=t2_[:], op=AOT.add)
                nc.vector.tensor_tensor(out=st2[:, 1:2], in0=qy[:], in1=t3_[:], op=AOT.add)
                # group-reduce (8 partitions per group) -> [16, 2]
                pg = pss.tile([16, 2], F32, tag="ps", space="PSUM")
                nc.tensor.matmul(pg[:], lhsT=sb_ggrp[:, :], rhs=st2[:],
                                 start=True, stop=True)
                nc.vector.tensor_copy(out=allst[:, :], in_=pg[:])
                # pair AllReduce of this o2-half's [16, 2] stats
                if with_collective:
                    bin_ = drpool.tile([16, 2], F32, tag="cin")
                    bout = drpool.tile([16, 2], F32, tag="cout")
                    nc.gpsimd.dma_start(out=bin_[:], in_=allst[:, :])
                    nc.gpsimd.collective_compute(
                        "AllReduce", AOT.add,
                        replica_groups=[[0, 1], [2, 3], [4, 5], [6, 7]],
                        ins=[bin_.opt()], outs=[bout.opt()])
                    nc.gpsimd.dma_start(out=allst2[:, :], in_=bout[:])
                else:
                    nc.vector.tensor_copy(out=allst2[:, :], in_=allst[:, :])

                # mean/rstd per group
                var = mth.tile([16, 1], F32, tag="v16")
                m2 = mth.tile([16, 1], F32, tag="v16")
                nc.vector.tensor_scalar(out=mr[:, 0:1], in0=allst2[:, 0:1],
                                        scalar1=1.0 / DIV, scalar2=None, op0=AOT.mult)
                nc.vector.tensor_tensor(out=m2[:], in0=mr[:, 0:1], in1=mr[:, 0:1],
                                        op=AOT.mult)
                nc.vector.tensor_scalar(out=var[:], in0=allst2[:, 1:2],
                                        scalar1=1.0 / DIV, scalar2=None, op0=AOT.mult)
                nc.vector.tensor_tensor(out=var[:], in0=var[:], in1=m2[:], op=AOT.subtract)
                nc.scalar.activation(out=var[:], in_=var[:], func=ACTF.Sqrt, bias=sb_eps[:])
                nc.vector.reciprocal(out=mr[:, 1:2], in_=var[:])

                pb = pss.tile([128, 2], F32, tag="ps", space="PSUM")
                nc.tensor.matmul(pb[:], lhsT=sb_gbrd[:, :], rhs=mr[:, :],
                                 start=True, stop=True)
                sc = mth.tile([128, 1], F32, tag="s1", bufs=12)
                bn = mth.tile([128, 1], F32, tag="s1", bufs=12)
                tb = mth.tile([128, 1], F32, tag="s1", bufs=12)
                nc.vector.tensor_tensor(out=sc[:], in0=sb_gam[:, o2:o2 + 1],
                                        in1=pb[:, 1:2], op=AOT.mult)
                nc.vector.tensor_tensor(out=tb[:], in0=sb_bvec[:, o2:o2 + 1],
                                        in1=pb[:, 0:1], op=AOT.subtract)
                nc.vector.tensor_tensor(out=tb[:], in0=sc[:], in1=tb[:], op=AOT.mult)
                nc.vector.tensor_tensor(out=bn[:], in0=tb[:], in1=sb_bet[:, o2:o2 + 1],
                                        op=AOT.add)
                for nt in range(4):
                    yo = yopool.tile([128, 512], F32, tag="yo")
                    nc.scalar.activation(out=yo[:], in_=ps_ys[nt][:],
                                         func=ACTF.Relu, scale=sc[:], bias=bn[:])
                    nc.sync.dma_start(out=yout[o2, :, nt * 512:(nt + 1) * 512], in_=yo[:])

    nc.compile()
    return nc


# ---------------------------------------------------------------------------
# host side
# ---------------------------------------------------------------------------
_NC_CACHE = {}


def get_nc(with_collective=True, debug_outs=False):
    key = (with_collective, debug_outs)
    if key not in _NC_CACHE:
        _NC_CACHE[key] = build_nc(with_collective, debug_outs)
    return _NC_CACHE[key]


def host_consts():
    p = np.arange(128)
    j = np.arange(16)
    t = np.arange(9)
    cxv = ((p % 64)[:, None, None] + (t % 3)[None, None, :] - 1 + SH) \
        + np.zeros((1, 16, 1))
    ggrp_ = (p[:, None] // 8 == np.arange(16)[None, :]).astype(np.float32)
    gbrd_ = (p[None, :] // 8 == np.arange(16)[:, None]).astype(np.float32)
    return cxv.astype(np.float32), ggrp_, gbrd_


def make_in_maps(x, w_off, b_off, w, b, gamma, beta):
    cxv, ggrp_, gbrd_ = host_consts()
    # weight layouts
    w4d = w.reshape(256, 2, 128, 3, 3)
    wt_ = np.ascontiguousarray(
        w4d.reshape(256, 2, 128, 9).transpose(2, 1, 3, 0)).astype(np.float32)
    wo4d = w_off.reshape(18, 2, 128, 9)
    wofft_ = np.ascontiguousarray(wo4d.transpose(2, 1, 3, 0)).astype(np.float32)
    bvec_ = np.ascontiguousarray(b.reshape(2, 128).T).astype(np.float32)
    gam_ = np.ascontiguousarray(gamma.reshape(2, 128).T).astype(np.float32)
    bet_ = np.ascontiguousarray(beta.reshape(2, 128).T).astype(np.float32)
    boff_ = b_off.reshape(18, 1).astype(np.float32)

    p = np.arange(128)
    j = np.arange(16)
    t = np.arange(9)
    in_maps = []
    for core in range(N_CORES):
        bb, half = core // 2, core % 2
        base = 32 * half
        xb = x[bb]                                   # [256, 64, 64]
        xt_ = np.ascontiguousarray(
            xb.reshape(256, 4096).T).astype(ml_dtypes.bfloat16)
        xc = np.zeros((2, 128, 34, 66), np.float32)
        r0, r1 = base - 1, base + 33                 # global rows [r0, r1)
        cr0, cr1 = max(r0, 0), min(r1, 64)
        xc[:, :, cr0 - r0: cr1 - r0, 1:65] = xb.reshape(2, 128, 64, 64)[:, :, cr0:cr1, :]
        cyv = (base + 2 * j[None, :, None] + (p // 64)[:, None, None]
               + (t // 3)[None, None, :] - 1 + SH).astype(np.float32)
        in_maps.append({
            "xt": xt_, "xconv": xc, "wofft": wofft_, "wt": wt_,
            "cy": cyv, "cx": cxv, "bvec": bvec_, "gamv": gam_, "betv": bet_,
            "boff": boff_, "ggrp": ggrp_, "gbrd": gbrd_,
        })
    return in_maps


def kernel(x, w_off, b_off, w, b, gamma, beta):
    nc = get_nc(with_collective=True)
    in_maps = make_in_maps(x, w_off, b_off, w, b, gamma, beta)
    res = run_bass_kernel_spmd(nc, in_maps, core_ids=list(range(N_CORES)))
    out = np.empty((B, C, H, W), np.float32)
    for core in range(N_CORES):
        bb, half = core // 2, core % 2
        yo = res.results[core]["yout"]               # [2, 128, 2048]
        out[bb, :, 32 * half:32 * half + 32, :] = yo.reshape(256, 32, 64)
    return out

